# revision 24
# baseline (speedup 1.0000x reference)
"""BoneLengthLoss Trainium2 kernel.

Full inputs: pose_3d_pred (524288, 37, 3) f32, pose_3d_ref same, valid_mask
(524288, 37) bool.  Output: scalar f32 = sum(sq_err * bone_valid) /
sum(bone_valid) over all (batch, bone) pairs.

Strategy: pure data-parallel over 8 NeuronCores (batch dim).  Per core:
65536 batch rows, processed in T tiles of 128 partitions x R rows each
(one batch row = 111 f32 = 37 kpts x 3).

Engine budget (per core, cost-model): DMA streams 60.6 MB of HBM
(~180 us floor — this is the target bound).  To get every other engine
under that floor:
  - input DMAs CAST f32 -> bf16 in flight (swdge/gpsimd DMAs support
    dtype conversion), so every DVE TensorTensor op runs in 2x mode
    (2-byte operands, unit-stride innermost dim);
  - DVE does the gathered bone diffs (13 run-APs), the triple-sum adds,
    len-diff and masked-error multiplies (~125 us);
  - ACT does the squares (3 contiguous c-planes; stride-3 reads are free
    on ACT) and the sqrt (~126 us);
  - Pool (gpsimd) does the mask gather-products (with den accum) and the
    final square+accumulate (num), plus swdge DMA dispatch (~75 us).
Each core returns per-partition partial (num, den); the host sums 8x128
partials and divides.
"""

import sys

sys.path.insert(0, "/opt/trn_rl_repo")

import numpy as np

# ---- problem constants (hardcoded; kernel.py must be self-contained) ----
N_CORES = 8
BATCH = 524288
KP = 37  # keypoints
NB = 32  # bones
B_CORE = BATCH // N_CORES  # 65536
P = 128  # SBUF partitions
R = 64  # batch rows per partition per tile
T = B_CORE // (P * R)  # tiles per core
RB = R * NB  # bone entries per partition per tile
ROW = KP * 3  # 111 floats per batch row

# Bone list decomposed into runs: (j1_start, s1, j2_start, s2, L).
# Bone i of a run connects joints (j1_start + i*s1, j2_start + i*s2).
# Output bone order = concatenation of runs (a permutation of the
# reference bone order — irrelevant, everything is summed).
RUNS = [
    (1, 0, 2, 1, 3),
    (2, 1, 5, 1, 2),
    (11, 0, 12, 1, 2),
    (12, 1, 14, 0, 2),
    (14, 1, 15, 1, 3),
    (12, 1, 18, 1, 2),
    (18, 1, 20, 1, 4),
    (16, 0, 24, 1, 2),
    (24, 1, 26, 0, 2),
    (24, 1, 27, 1, 2),
    (27, 1, 29, 1, 4),
    (17, 1, 33, 16, 1),
    (33, 1, 34, 1, 3),
]
assert sum(r[4] for r in RUNS) == NB

# Run-index groups for the mask (bone_valid) instructions: pairs share one
# instruction via a raw-AP pair dimension; members must have equal
# (s1 == 0, s2 == 0, L) signature.
MV_GROUPS = [(2, 7), (3, 8), (4, 12), (6, 10), (1, 5), (0,), (9,), (11,)]
_cov = sorted(i for g in MV_GROUPS for i in g)
assert _cov == list(range(len(RUNS)))
for g in MV_GROUPS:
    if len(g) == 2:
        a, b = RUNS[g[0]], RUNS[g[1]]
        assert a[1] == b[1] and a[3] == b[3] and a[4] == b[4]

_COMPILED = None


def _build(T=T):
    from concourse import bacc, bass, tile
    import concourse.mybir as mybir

    f32 = mybir.dt.float32
    u8 = mybir.dt.uint8
    DT = mybir.dt.bfloat16  # on-chip dtype (DMA casts f32 -> DT)

    nc = bacc.Bacc("TRN2", target_bir_lowering=False, debug=False)

    pred_d = nc.dram_tensor("pred", [T, P, R * ROW], f32, kind="ExternalInput")
    ref_d = nc.dram_tensor("ref", [T, P, R * ROW], f32, kind="ExternalInput")
    mask_d = nc.dram_tensor("mask", [T, P, R * KP], u8, kind="ExternalInput")
    eye_d = nc.dram_tensor("eye", [P, P], DT, kind="ExternalInput")
    out_d = nc.dram_tensor("out", [P, 2], f32, kind="ExternalOutput")

    with tile.TileContext(nc) as tc:
        with (
            tc.tile_pool(name="io", bufs=2) as io_pool,
            tc.tile_pool(name="work", bufs=2) as work_pool,
            tc.tile_pool(name="acc", bufs=1) as acc_pool,
            tc.tile_pool(name="psum", bufs=2, space=bass.MemorySpace.PSUM) as psum_pool,
        ):
            # Chunk list: first DRAM tile split into quarters (pipeline
            # ramp-up starts after ~1/4 tile of DMA), last tile split into
            # halves (shorter serial drain chain).
            chunks = []
            for s in range(4):
                chunks.append((0, s * (R // 4), R // 4))
            for t in range(1, T - 1):
                chunks.append((t, 0, R))
            for s in range(2):
                chunks.append((T - 1, s * (R // 2), R // 2))
            NCH = len(chunks)

            eye = acc_pool.tile([P, P], DT)

            NG = len(RUNS)
            numstrip = acc_pool.tile([P, NCH], f32)
            denwide = acc_pool.tile([P, NCH * NG], f32)

            # run start positions in the output bone order
            pos_of = []
            pos = 0
            for j1, s1, j2, s2, L in RUNS:
                pos_of.append(pos)
                pos += L

            for ci, (t, roff, R_) in enumerate(chunks):
                RB_ = R_ * NB
                pq = io_pool.tile([P, 2 * R_ * ROW], DT, tag="pq", bufs=3)
                m8 = io_pool.tile([P, R_ * KP], u8, tag="m8")
                # mask first: the MV products depend only on it and can
                # run while the (much larger) pose DMAs stream.
                # casting pose DMAs (f32 -> bf16): swdge only, hence gpsimd
                nc.gpsimd.dma_start(
                    m8[:], mask_d[t][:, roff * KP : (roff + R_) * KP]
                )
                nc.gpsimd.dma_start(
                    pq[:, : R_ * ROW],
                    pred_d[t][:, roff * ROW : (roff + R_) * ROW],
                )
                nc.gpsimd.dma_start(
                    pq[:, R_ * ROW :],
                    ref_d[t][:, roff * ROW : (roff + R_) * ROW],
                )
                if ci == 0:
                    # eye is first needed by the chunk-0 matmuls; dispatch
                    # after chunk 0's input DMAs so they land earlier.
                    nc.gpsimd.dma_start(eye[:], eye_d[:])

                # D: bone differences, interleaved [p][pose(2)][r][b(32)][c(3)]
                # so every diff write lands contiguously.  All operands bf16
                # with unit-stride innermost (c) dim -> DVE 2x mode.
                D = work_pool.tile([P, 6 * RB_], DT, tag="D")
                pqv = pq.rearrange(
                    "p (g r k c) -> p g r k c", g=2, r=R_, k=KP, c=3
                )
                Dv = D.rearrange(
                    "p (g r b c) -> p g r b c", g=2, r=R_, b=NB, c=3
                )
                for ri, (j1, s1, j2, s2, L) in enumerate(RUNS):
                    # 5-dim APs: both poses in one instruction
                    if s1 == 0:
                        a1 = pqv[:, :, :, j1 : j1 + 1, :].to_broadcast(
                            [P, 2, R_, L, 3]
                        )
                    else:
                        a1 = pqv[:, :, :, j1 : j1 + s1 * (L - 1) + 1 : s1, :]
                    if s2 == 0:
                        a2 = pqv[:, :, :, j2 : j2 + 1, :].to_broadcast(
                            [P, 2, R_, L, 3]
                        )
                    else:
                        a2 = pqv[:, :, :, j2 : j2 + s2 * (L - 1) + 1 : s2, :]
                    o = Dv[:, :, :, pos_of[ri] : pos_of[ri] + L, :]
                    # pred_vec = pose[J2] - pose[J1].  (Broadcast runs cost
                    # 1x on DVE; running them on Pool instead was tried and
                    # lost — Pool compute contends for SBUF bandwidth and
                    # slowed every DVE op by 20-50%.)
                    nc.vector.tensor_sub(o, a2, a1)

                # squares on ACT, written as three c-split planes (stride-3
                # reads are ~free on ACT; contiguous writes).
                SQ = work_pool.tile([P, 6 * RB_], DT, tag="SQ")
                Dq = D.rearrange("p (q c) -> p q c", q=2 * RB_, c=3)
                for k in range(3):
                    nc.scalar.activation(
                        SQ[:, k * 2 * RB_ : (k + 1) * 2 * RB_],
                        Dq[:, :, k],
                        mybir.ActivationFunctionType.Square,
                    )
                # Triple-sum on the (otherwise idle) PE: three accumulating
                # identity matmuls per 512-column PSUM bank add the three
                # planes in fp32 PSUM, freeing the DVE adds.  Processed in
                # two PSUM halves (bufs=2) so chunk N+1's matmuls overlap
                # chunk N's sqrt instead of serializing on one PSUM tile.
                # The lengths overwrite the head of D (dead after the
                # squares) — the freed SBUF pays for pq bufs=3.
                for h in range(2):
                    ps = psum_pool.tile([P, RB_], f32, tag="ps")
                    hoff = h * RB_
                    for j in range(RB_ // 512):
                        lo, hi = j * 512, (j + 1) * 512
                        for k in range(3):
                            nc.tensor.matmul(
                                ps[:, lo:hi],
                                eye[:],
                                SQ[:, k * 2 * RB_ + hoff + lo : k * 2 * RB_ + hoff + hi],
                                start=(k == 0),
                                stop=(k == 2),
                            )
                    # lengths: sqrt PSUM f32 -> SBUF bf16 (into D's head)
                    nc.scalar.sqrt(D[:, hoff : hoff + RB_], ps[:])

                # E = pred_len - ref_len  (bf16, 2x)
                E = D[:, :RB_]
                nc.vector.tensor_sub(E, D[:, :RB_], D[:, RB_ : 2 * RB_])

                # bone_valid = mask[J1] * mask[J2]  (u8 -> DT).  The Pool
                # engine can't run TensorScalarPtr (ISA check), so these
                # stay on DVE (1x: u8 operands).  accum_out fuses the den
                # partial sum.
                MV = work_pool.tile([P, RB_], DT, tag="MV")
                m8t = m8[:].tensor
                mvt = MV[:].tensor
                mp = R_ * KP  # m8 partition pitch (elements)
                vp = RB_  # MV partition pitch
                for ri, (j1, s1, j2, s2, L) in enumerate(RUNS):
                    posa = pos_of[ri]
                    b1 = bass.AP(m8t, j1, [[mp, P], [KP, R_], [s1, L]])
                    b2 = bass.AP(m8t, j2, [[mp, P], [KP, R_], [s2, L]])
                    o = bass.AP(mvt, posa, [[vp, P], [NB, R_], [1, L]])
                    nc.vector.scalar_tensor_tensor(
                        out=o,
                        in0=b1,
                        scalar=1.0,
                        in1=b2,
                        op0=mybir.AluOpType.bypass,
                        op1=mybir.AluOpType.mult,
                        accum_out=denwide[:, ci * NG + ri : ci * NG + ri + 1],
                    )

                # masked error on DVE (bf16, 2x); ACT does square+row-sum
                # (num) via activation accumulate
                ME = D[:, RB_ : 2 * RB_]
                nc.vector.tensor_tensor(ME, E, MV[:], op=mybir.AluOpType.mult)
                nc.scalar.activation(
                    ME,
                    ME,
                    mybir.ActivationFunctionType.Square,
                    accum_out=numstrip[:, ci : ci + 1],
                )

            acc2 = acc_pool.tile([P, 2], f32)
            nc.vector.reduce_sum(acc2[:, 0:1], numstrip[:], axis=mybir.AxisListType.X)
            nc.vector.reduce_sum(acc2[:, 1:2], denwide[:], axis=mybir.AxisListType.X)
            nc.gpsimd.dma_start(out_d[:], acc2[:])

    nc.compile()
    return nc


def _get_nc():
    global _COMPILED
    if _COMPILED is None:
        _COMPILED = _build()
    return _COMPILED


def _make_in_maps(pose_3d_pred, pose_3d_ref, valid_mask):
    import ml_dtypes

    pred = np.ascontiguousarray(np.asarray(pose_3d_pred, dtype=np.float32))
    ref = np.ascontiguousarray(np.asarray(pose_3d_ref, dtype=np.float32))
    mask = np.ascontiguousarray(np.asarray(valid_mask)).astype(np.uint8)
    eye = np.eye(P, dtype=ml_dtypes.bfloat16)
    in_maps = []
    for c in range(N_CORES):
        sl = slice(c * B_CORE, (c + 1) * B_CORE)
        in_maps.append(
            {
                "pred": pred[sl].reshape(T, P, R * ROW),
                "ref": ref[sl].reshape(T, P, R * ROW),
                "mask": mask[sl].reshape(T, P, R * KP),
                "eye": eye,
            }
        )
    return in_maps


def kernel(pose_3d_pred, pose_3d_ref, valid_mask, _trace=False):
    from concourse.bass_utils import run_bass_kernel_spmd

    nc = _get_nc()
    in_maps = _make_in_maps(pose_3d_pred, pose_3d_ref, valid_mask)
    res = run_bass_kernel_spmd(nc, in_maps, list(range(N_CORES)), trace=_trace)
    num = 0.0
    den = 0.0
    for i in range(N_CORES):
        o = res.results[i]["out"].astype(np.float64)
        num += o[:, 0].sum()
        den += o[:, 1].sum()
    out = np.float32(num / den)
    if _trace:
        return out, res
    return out


# revision 27
# speedup vs baseline: 1.0200x; 1.0200x over previous
"""BoneLengthLoss Trainium2 kernel.

Full inputs: pose_3d_pred (524288, 37, 3) f32, pose_3d_ref same, valid_mask
(524288, 37) bool.  Output: scalar f32 = sum(sq_err * bone_valid) /
sum(bone_valid) over all (batch, bone) pairs.

Strategy: pure data-parallel over 8 NeuronCores (batch dim).  Per core:
65536 batch rows, processed in T tiles of 128 partitions x R rows each
(one batch row = 111 f32 = 37 kpts x 3).

Engine budget (per core, cost-model): DMA streams 60.6 MB of HBM
(~180 us floor — this is the target bound).  To get every other engine
under that floor:
  - input DMAs CAST f32 -> bf16 in flight (swdge/gpsimd DMAs support
    dtype conversion), so every DVE TensorTensor op runs in 2x mode
    (2-byte operands, unit-stride innermost dim);
  - DVE does the gathered bone diffs (13 run-APs), the triple-sum adds,
    len-diff and masked-error multiplies (~125 us);
  - ACT does the squares (3 contiguous c-planes; stride-3 reads are free
    on ACT) and the sqrt (~126 us);
  - Pool (gpsimd) does the mask gather-products (with den accum) and the
    final square+accumulate (num), plus swdge DMA dispatch (~75 us).
Each core returns per-partition partial (num, den); the host sums 8x128
partials and divides.
"""

import sys

sys.path.insert(0, "/opt/trn_rl_repo")

import numpy as np

# ---- problem constants (hardcoded; kernel.py must be self-contained) ----
N_CORES = 8
BATCH = 524288
KP = 37  # keypoints
NB = 32  # bones
B_CORE = BATCH // N_CORES  # 65536
P = 128  # SBUF partitions
R = 64  # batch rows per partition per tile
T = B_CORE // (P * R)  # tiles per core
RB = R * NB  # bone entries per partition per tile
ROW = KP * 3  # 111 floats per batch row

# Bone list decomposed into runs: (j1_start, s1, j2_start, s2, L).
# Bone i of a run connects joints (j1_start + i*s1, j2_start + i*s2).
# Output bone order = concatenation of runs (a permutation of the
# reference bone order — irrelevant, everything is summed).
RUNS = [
    (1, 0, 2, 1, 3),
    (2, 1, 5, 1, 2),
    (11, 0, 12, 1, 2),
    (12, 1, 14, 0, 2),
    (14, 1, 15, 1, 3),
    (12, 1, 18, 1, 2),
    (18, 1, 20, 1, 4),
    (16, 0, 24, 1, 2),
    (24, 1, 26, 0, 2),
    (24, 1, 27, 1, 2),
    (27, 1, 29, 1, 4),
    (17, 1, 33, 16, 1),
    (33, 1, 34, 1, 3),
]
assert sum(r[4] for r in RUNS) == NB

# Run-index groups for the mask (bone_valid) instructions: pairs share one
# instruction via a raw-AP pair dimension; members must have equal
# (s1 == 0, s2 == 0, L) signature.
MV_GROUPS = [(2, 7), (3, 8), (4, 12), (6, 10), (1, 5), (0,), (9,), (11,)]
_cov = sorted(i for g in MV_GROUPS for i in g)
assert _cov == list(range(len(RUNS)))
for g in MV_GROUPS:
    if len(g) == 2:
        a, b = RUNS[g[0]], RUNS[g[1]]
        assert a[1] == b[1] and a[3] == b[3] and a[4] == b[4]

_COMPILED = None


def _build(T=T):
    from concourse import bacc, bass, tile
    import concourse.mybir as mybir

    f32 = mybir.dt.float32
    u8 = mybir.dt.uint8
    DT = mybir.dt.bfloat16  # on-chip dtype (DMA casts f32 -> DT)

    nc = bacc.Bacc("TRN2", target_bir_lowering=False, debug=False)

    pred_d = nc.dram_tensor("pred", [T, P, R * ROW], f32, kind="ExternalInput")
    ref_d = nc.dram_tensor("ref", [T, P, R * ROW], f32, kind="ExternalInput")
    mask_d = nc.dram_tensor("mask", [T, P, R * KP], u8, kind="ExternalInput")
    eye_d = nc.dram_tensor("eye", [P, P], DT, kind="ExternalInput")
    out_d = nc.dram_tensor("out", [P, 2], f32, kind="ExternalOutput")

    with tile.TileContext(nc) as tc:
        with (
            tc.tile_pool(name="io", bufs=2) as io_pool,
            tc.tile_pool(name="work", bufs=2) as work_pool,
            tc.tile_pool(name="acc", bufs=1) as acc_pool,
            tc.tile_pool(name="psum", bufs=2, space=bass.MemorySpace.PSUM) as psum_pool,
        ):
            # Chunk list: first DRAM tile split into quarters (pipeline
            # ramp-up starts after ~1/4 tile of DMA), last tile split into
            # halves (shorter serial drain chain).
            chunks = []
            for s in range(4):
                chunks.append((0, s * (R // 4), R // 4))
            for t in range(1, T - 1):
                chunks.append((t, 0, R))
            for s in range(2):
                chunks.append((T - 1, s * (R // 2), R // 2))
            NCH = len(chunks)

            eye = acc_pool.tile([P, P], DT)

            NG = len(RUNS)
            numstrip = acc_pool.tile([P, NCH], f32)
            denwide = acc_pool.tile([P, NCH * NG], f32)

            # run start positions in the output bone order
            pos_of = []
            pos = 0
            for j1, s1, j2, s2, L in RUNS:
                pos_of.append(pos)
                pos += L

            for ci, (t, roff, R_) in enumerate(chunks):
                RB_ = R_ * NB
                pq = io_pool.tile([P, 2 * R_ * ROW], DT, tag="pq")
                m8 = io_pool.tile([P, R_ * KP], u8, tag="m8")
                # mask first: the MV products depend only on it and can
                # run while the (much larger) pose DMAs stream.
                # casting pose DMAs (f32 -> bf16): swdge only, hence gpsimd
                nc.gpsimd.dma_start(
                    m8[:], mask_d[t][:, roff * KP : (roff + R_) * KP]
                )
                nc.gpsimd.dma_start(
                    pq[:, : R_ * ROW],
                    pred_d[t][:, roff * ROW : (roff + R_) * ROW],
                )
                nc.gpsimd.dma_start(
                    pq[:, R_ * ROW :],
                    ref_d[t][:, roff * ROW : (roff + R_) * ROW],
                )
                if ci == 0:
                    # eye is first needed by the chunk-0 matmuls; dispatch
                    # after chunk 0's input DMAs so they land earlier.
                    nc.gpsimd.dma_start(eye[:], eye_d[:])

                # bone_valid = mask[J1] * mask[J2]  (u8 -> DT).  The Pool
                # engine can't run TensorScalarPtr (ISA check), so these
                # stay on DVE (1x: u8 operands).  accum_out fuses the den
                # partial sum.  Emitted BEFORE the diffs: MV depends only
                # on the (tiny, early) mask DMA, so the scheduler can slot
                # it into pose-DMA wait stalls.
                MV = work_pool.tile([P, RB_], DT, tag="MV")
                m8t = m8[:].tensor
                mvt = MV[:].tensor
                mp = R_ * KP  # m8 partition pitch (elements)
                vp = RB_  # MV partition pitch
                for ri, (j1, s1, j2, s2, L) in enumerate(RUNS):
                    posa = pos_of[ri]
                    b1 = bass.AP(m8t, j1, [[mp, P], [KP, R_], [s1, L]])
                    b2 = bass.AP(m8t, j2, [[mp, P], [KP, R_], [s2, L]])
                    o = bass.AP(mvt, posa, [[vp, P], [NB, R_], [1, L]])
                    nc.vector.scalar_tensor_tensor(
                        out=o,
                        in0=b1,
                        scalar=1.0,
                        in1=b2,
                        op0=mybir.AluOpType.bypass,
                        op1=mybir.AluOpType.mult,
                        accum_out=denwide[:, ci * NG + ri : ci * NG + ri + 1],
                    )

                # D: bone differences, interleaved [p][pose(2)][r][b(32)][c(3)]
                # so every diff write lands contiguously.  All operands bf16
                # with unit-stride innermost (c) dim -> DVE 2x mode.
                D = work_pool.tile([P, 6 * RB_], DT, tag="D")
                pqv = pq.rearrange(
                    "p (g r k c) -> p g r k c", g=2, r=R_, k=KP, c=3
                )
                Dv = D.rearrange(
                    "p (g r b c) -> p g r b c", g=2, r=R_, b=NB, c=3
                )
                for ri, (j1, s1, j2, s2, L) in enumerate(RUNS):
                    # 5-dim APs: both poses in one instruction
                    if s1 == 0:
                        a1 = pqv[:, :, :, j1 : j1 + 1, :].to_broadcast(
                            [P, 2, R_, L, 3]
                        )
                    else:
                        a1 = pqv[:, :, :, j1 : j1 + s1 * (L - 1) + 1 : s1, :]
                    if s2 == 0:
                        a2 = pqv[:, :, :, j2 : j2 + 1, :].to_broadcast(
                            [P, 2, R_, L, 3]
                        )
                    else:
                        a2 = pqv[:, :, :, j2 : j2 + s2 * (L - 1) + 1 : s2, :]
                    o = Dv[:, :, :, pos_of[ri] : pos_of[ri] + L, :]
                    # pred_vec = pose[J2] - pose[J1].  (Broadcast runs cost
                    # 1x on DVE; running them on Pool instead was tried and
                    # lost — Pool compute contends for SBUF bandwidth and
                    # slowed every DVE op by 20-50%.)
                    nc.vector.tensor_sub(o, a2, a1)

                # squares on ACT, written as three c-split planes (stride-3
                # reads are ~free on ACT; contiguous writes).
                SQ = work_pool.tile([P, 6 * RB_], DT, tag="SQ")
                Dq = D.rearrange("p (q c) -> p q c", q=2 * RB_, c=3)
                for k in range(3):
                    nc.scalar.activation(
                        SQ[:, k * 2 * RB_ : (k + 1) * 2 * RB_],
                        Dq[:, :, k],
                        mybir.ActivationFunctionType.Square,
                    )
                # Triple-sum on the (otherwise idle) PE: three accumulating
                # identity matmuls per 512-column PSUM bank add the three
                # planes in fp32 PSUM, freeing the DVE adds.  Processed in
                # two PSUM halves (bufs=2) so chunk N+1's matmuls overlap
                # chunk N's sqrt instead of serializing on one PSUM tile.
                L2 = work_pool.tile([P, 2 * RB_], DT, tag="L2")
                for h in range(2):
                    ps = psum_pool.tile([P, RB_], f32, tag="ps")
                    hoff = h * RB_
                    for j in range(RB_ // 512):
                        lo, hi = j * 512, (j + 1) * 512
                        for k in range(3):
                            nc.tensor.matmul(
                                ps[:, lo:hi],
                                eye[:],
                                SQ[:, k * 2 * RB_ + hoff + lo : k * 2 * RB_ + hoff + hi],
                                start=(k == 0),
                                stop=(k == 2),
                            )
                    # lengths: sqrt PSUM f32 -> SBUF bf16
                    nc.scalar.sqrt(L2[:, hoff : hoff + RB_], ps[:])

                # E = pred_len - ref_len  (bf16, 2x)
                E = L2[:, :RB_]
                nc.vector.tensor_sub(E, L2[:, :RB_], L2[:, RB_ : 2 * RB_])

                # masked error on DVE (bf16, 2x); ACT does square+row-sum
                # (num) via activation accumulate
                ME = L2[:, RB_ : 2 * RB_]
                nc.vector.tensor_tensor(ME, E, MV[:], op=mybir.AluOpType.mult)
                nc.scalar.activation(
                    ME,
                    ME,
                    mybir.ActivationFunctionType.Square,
                    accum_out=numstrip[:, ci : ci + 1],
                )

            acc2 = acc_pool.tile([P, 2], f32)
            nc.vector.reduce_sum(acc2[:, 0:1], numstrip[:], axis=mybir.AxisListType.X)
            nc.vector.reduce_sum(acc2[:, 1:2], denwide[:], axis=mybir.AxisListType.X)
            nc.gpsimd.dma_start(out_d[:], acc2[:])

    nc.compile()
    return nc


def _get_nc():
    global _COMPILED
    if _COMPILED is None:
        _COMPILED = _build()
    return _COMPILED


def _make_in_maps(pose_3d_pred, pose_3d_ref, valid_mask):
    import ml_dtypes

    pred = np.ascontiguousarray(np.asarray(pose_3d_pred, dtype=np.float32))
    ref = np.ascontiguousarray(np.asarray(pose_3d_ref, dtype=np.float32))
    mask = np.ascontiguousarray(np.asarray(valid_mask)).astype(np.uint8)
    eye = np.eye(P, dtype=ml_dtypes.bfloat16)
    in_maps = []
    for c in range(N_CORES):
        sl = slice(c * B_CORE, (c + 1) * B_CORE)
        in_maps.append(
            {
                "pred": pred[sl].reshape(T, P, R * ROW),
                "ref": ref[sl].reshape(T, P, R * ROW),
                "mask": mask[sl].reshape(T, P, R * KP),
                "eye": eye,
            }
        )
    return in_maps


def kernel(pose_3d_pred, pose_3d_ref, valid_mask, _trace=False):
    from concourse.bass_utils import run_bass_kernel_spmd

    nc = _get_nc()
    in_maps = _make_in_maps(pose_3d_pred, pose_3d_ref, valid_mask)
    res = run_bass_kernel_spmd(nc, in_maps, list(range(N_CORES)), trace=_trace)
    num = 0.0
    den = 0.0
    for i in range(N_CORES):
        o = res.results[i]["out"].astype(np.float64)
        num += o[:, 0].sum()
        den += o[:, 1].sum()
    out = np.float32(num / den)
    if _trace:
        return out, res
    return out


# revision 29
# speedup vs baseline: 1.1573x; 1.1346x over previous
"""BoneLengthLoss Trainium2 kernel.

Full inputs: pose_3d_pred (524288, 37, 3) f32, pose_3d_ref same, valid_mask
(524288, 37) bool.  Output: scalar f32 = sum(sq_err * bone_valid) /
sum(bone_valid) over all (batch, bone) pairs.

Strategy: pure data-parallel over 8 NeuronCores (batch dim).  Per core:
65536 batch rows, processed in T tiles of 128 partitions x R rows each
(one batch row = 111 f32 = 37 kpts x 3).

Engine budget (per core, cost-model): DMA streams 60.6 MB of HBM
(~180 us floor — this is the target bound).  To get every other engine
under that floor:
  - input DMAs CAST f32 -> bf16 in flight (swdge/gpsimd DMAs support
    dtype conversion), so every DVE TensorTensor op runs in 2x mode
    (2-byte operands, unit-stride innermost dim);
  - DVE does the gathered bone diffs (13 run-APs), the triple-sum adds,
    len-diff and masked-error multiplies (~125 us);
  - ACT does the squares (3 contiguous c-planes; stride-3 reads are free
    on ACT) and the sqrt (~126 us);
  - Pool (gpsimd) does the mask gather-products (with den accum) and the
    final square+accumulate (num), plus swdge DMA dispatch (~75 us).
Each core returns per-partition partial (num, den); the host sums 8x128
partials and divides.
"""

import sys

sys.path.insert(0, "/opt/trn_rl_repo")

import numpy as np

# ---- problem constants (hardcoded; kernel.py must be self-contained) ----
N_CORES = 8
BATCH = 524288
KP = 37  # keypoints
NB = 32  # bones
B_CORE = BATCH // N_CORES  # 65536
P = 128  # SBUF partitions
R = 64  # batch rows per partition per tile
T = B_CORE // (P * R)  # tiles per core
RB = R * NB  # bone entries per partition per tile
ROW = KP * 3  # 111 floats per batch row

# Bone list decomposed into runs: (j1_start, s1, j2_start, s2, L).
# Bone i of a run connects joints (j1_start + i*s1, j2_start + i*s2).
# Output bone order = concatenation of runs (a permutation of the
# reference bone order — irrelevant, everything is summed).
RUNS = [
    (1, 0, 2, 1, 3),
    (2, 1, 5, 1, 2),
    (11, 0, 12, 1, 2),
    (12, 1, 14, 0, 2),
    (14, 1, 15, 1, 3),
    (12, 1, 18, 1, 2),
    (18, 1, 20, 1, 4),
    (16, 0, 24, 1, 2),
    (24, 1, 26, 0, 2),
    (24, 1, 27, 1, 2),
    (27, 1, 29, 1, 4),
    (17, 1, 33, 16, 1),
    (33, 1, 34, 1, 3),
]
assert sum(r[4] for r in RUNS) == NB

# Run-index groups for the mask (bone_valid) instructions: pairs share one
# instruction via a raw-AP pair dimension; members must have equal
# (s1 == 0, s2 == 0, L) signature.
MV_GROUPS = [(2, 7), (3, 8), (4, 12), (6, 10), (1, 5), (0,), (9,), (11,)]
_cov = sorted(i for g in MV_GROUPS for i in g)
assert _cov == list(range(len(RUNS)))
for g in MV_GROUPS:
    if len(g) == 2:
        a, b = RUNS[g[0]], RUNS[g[1]]
        assert a[1] == b[1] and a[3] == b[3] and a[4] == b[4]

_COMPILED = None


def _build(T=T):
    from concourse import bacc, bass, tile
    import concourse.mybir as mybir

    f32 = mybir.dt.float32
    u8 = mybir.dt.uint8
    DT = mybir.dt.bfloat16  # on-chip dtype (DMA casts f32 -> DT)

    nc = bacc.Bacc("TRN2", target_bir_lowering=False, debug=False)

    pred_d = nc.dram_tensor("pred", [T, P, R * ROW], f32, kind="ExternalInput")
    ref_d = nc.dram_tensor("ref", [T, P, R * ROW], f32, kind="ExternalInput")
    mask_d = nc.dram_tensor("mask", [T, P, R * KP], u8, kind="ExternalInput")
    eye_d = nc.dram_tensor("eye", [P, P], DT, kind="ExternalInput")
    out_d = nc.dram_tensor("out", [P, 2], f32, kind="ExternalOutput")

    with tile.TileContext(nc) as tc:
        with (
            tc.tile_pool(name="io", bufs=2) as io_pool,
            tc.tile_pool(name="work", bufs=2) as work_pool,
            tc.tile_pool(name="acc", bufs=1) as acc_pool,
            tc.tile_pool(name="psum", bufs=2, space=bass.MemorySpace.PSUM) as psum_pool,
        ):
            # Chunk list: first DRAM tile split into quarters (pipeline
            # ramp-up starts after ~1/4 tile of DMA), last tile split into
            # halves (shorter serial drain chain).
            chunks = []
            for s in range(4):
                chunks.append((0, s * (R // 4), R // 4))
            for t in range(1, T - 1):
                chunks.append((t, 0, R))
            for s in range(2):
                chunks.append((T - 1, s * (R // 2), R // 2))
            NCH = len(chunks)

            eye = acc_pool.tile([P, P], DT)

            NG = len(RUNS)
            numstrip = acc_pool.tile([P, NCH], f32)
            denwide = acc_pool.tile([P, NCH * NG], f32)

            # run start positions in the output bone order
            pos_of = []
            pos = 0
            for j1, s1, j2, s2, L in RUNS:
                pos_of.append(pos)
                pos += L

            for ci, (t, roff, R_) in enumerate(chunks):
                RB_ = R_ * NB
                pq = io_pool.tile([P, 2 * R_ * ROW], DT, tag="pq")
                m8 = io_pool.tile([P, R_ * KP], u8, tag="m8")
                # mask first: the MV products depend only on it and can
                # run while the (much larger) pose DMAs stream.
                # casting pose DMAs (f32 -> bf16): swdge only, hence gpsimd
                nc.gpsimd.dma_start(
                    m8[:], mask_d[t][:, roff * KP : (roff + R_) * KP]
                )
                nc.gpsimd.dma_start(
                    pq[:, : R_ * ROW],
                    pred_d[t][:, roff * ROW : (roff + R_) * ROW],
                )
                nc.gpsimd.dma_start(
                    pq[:, R_ * ROW :],
                    ref_d[t][:, roff * ROW : (roff + R_) * ROW],
                )
                if ci == 0:
                    # eye is first needed by the chunk-0 matmuls; dispatch
                    # after chunk 0's input DMAs so they land earlier.
                    nc.gpsimd.dma_start(eye[:], eye_d[:])

                # D: bone differences, interleaved [p][pose(2)][r][b(32)][c(3)]
                # so every diff write lands contiguously.  All operands bf16
                # with unit-stride innermost (c) dim -> DVE 2x mode.
                D = work_pool.tile([P, 6 * RB_], DT, tag="D")
                pqv = pq.rearrange(
                    "p (g r k c) -> p g r k c", g=2, r=R_, k=KP, c=3
                )
                Dv = D.rearrange(
                    "p (g r b c) -> p g r b c", g=2, r=R_, b=NB, c=3
                )
                for ri, (j1, s1, j2, s2, L) in enumerate(RUNS):
                    # 5-dim APs: both poses in one instruction
                    if s1 == 0:
                        a1 = pqv[:, :, :, j1 : j1 + 1, :].to_broadcast(
                            [P, 2, R_, L, 3]
                        )
                    else:
                        a1 = pqv[:, :, :, j1 : j1 + s1 * (L - 1) + 1 : s1, :]
                    if s2 == 0:
                        a2 = pqv[:, :, :, j2 : j2 + 1, :].to_broadcast(
                            [P, 2, R_, L, 3]
                        )
                    else:
                        a2 = pqv[:, :, :, j2 : j2 + s2 * (L - 1) + 1 : s2, :]
                    o = Dv[:, :, :, pos_of[ri] : pos_of[ri] + L, :]
                    # pred_vec = pose[J2] - pose[J1].  (Broadcast runs cost
                    # 1x on DVE; running them on Pool instead was tried and
                    # lost — Pool compute contends for SBUF bandwidth and
                    # slowed every DVE op by 20-50%.)
                    nc.vector.tensor_sub(o, a2, a1)

                # squares on ACT, written as three c-split planes (stride-3
                # reads are ~free on ACT; contiguous writes).
                SQ = work_pool.tile([P, 6 * RB_], DT, tag="SQ")
                Dq = D.rearrange("p (q c) -> p q c", q=2 * RB_, c=3)
                for k in range(3):
                    nc.scalar.activation(
                        SQ[:, k * 2 * RB_ : (k + 1) * 2 * RB_],
                        Dq[:, :, k],
                        mybir.ActivationFunctionType.Square,
                    )
                # Triple-sum on the (otherwise idle) PE: three accumulating
                # identity matmuls per 512-column PSUM bank add the three
                # planes in fp32 PSUM, freeing the DVE adds.  Processed in
                # two PSUM halves (bufs=2) so chunk N+1's matmuls overlap
                # chunk N's sqrt instead of serializing on one PSUM tile.
                L2 = work_pool.tile([P, 2 * RB_], DT, tag="L2")
                for h in range(2):
                    ps = psum_pool.tile([P, RB_], f32, tag="ps")
                    hoff = h * RB_
                    for j in range(RB_ // 512):
                        lo, hi = j * 512, (j + 1) * 512
                        for k in range(3):
                            nc.tensor.matmul(
                                ps[:, lo:hi],
                                eye[:],
                                SQ[:, k * 2 * RB_ + hoff + lo : k * 2 * RB_ + hoff + hi],
                                start=(k == 0),
                                stop=(k == 2),
                            )
                    # lengths: sqrt PSUM f32 -> SBUF bf16
                    nc.scalar.sqrt(L2[:, hoff : hoff + RB_], ps[:])

                # E = pred_len - ref_len  (bf16, 2x)
                E = L2[:, :RB_]
                nc.vector.tensor_sub(E, L2[:, :RB_], L2[:, RB_ : 2 * RB_])

                # bone_valid = mask[J1] * mask[J2]  (u8 -> DT).  The Pool
                # engine can't run TensorScalarPtr (ISA check), so these
                # stay on DVE (1x: u8 operands).  accum_out fuses the den
                # partial sum.
                MV = work_pool.tile([P, RB_], DT, tag="MV")
                m8t = m8[:].tensor
                mvt = MV[:].tensor
                mp = R_ * KP  # m8 partition pitch (elements)
                vp = RB_  # MV partition pitch
                for ri, (j1, s1, j2, s2, L) in enumerate(RUNS):
                    posa = pos_of[ri]
                    b1 = bass.AP(m8t, j1, [[mp, P], [KP, R_], [s1, L]])
                    b2 = bass.AP(m8t, j2, [[mp, P], [KP, R_], [s2, L]])
                    o = bass.AP(mvt, posa, [[vp, P], [NB, R_], [1, L]])
                    nc.vector.scalar_tensor_tensor(
                        out=o,
                        in0=b1,
                        scalar=1.0,
                        in1=b2,
                        op0=mybir.AluOpType.bypass,
                        op1=mybir.AluOpType.mult,
                        accum_out=denwide[:, ci * NG + ri : ci * NG + ri + 1],
                    )

                # masked error on DVE (bf16, 2x); ACT does square+row-sum
                # (num) via activation accumulate
                ME = L2[:, RB_ : 2 * RB_]
                nc.vector.tensor_tensor(ME, E, MV[:], op=mybir.AluOpType.mult)
                nc.scalar.activation(
                    ME,
                    ME,
                    mybir.ActivationFunctionType.Square,
                    accum_out=numstrip[:, ci : ci + 1],
                )

            acc2 = acc_pool.tile([P, 2], f32)
            nc.vector.reduce_sum(acc2[:, 0:1], numstrip[:], axis=mybir.AxisListType.X)
            nc.vector.reduce_sum(acc2[:, 1:2], denwide[:], axis=mybir.AxisListType.X)
            nc.gpsimd.dma_start(out_d[:], acc2[:])

    nc.compile()
    return nc


def _get_nc():
    global _COMPILED
    if _COMPILED is None:
        _COMPILED = _build()
    return _COMPILED


def _make_in_maps(pose_3d_pred, pose_3d_ref, valid_mask):
    import ml_dtypes

    pred = np.ascontiguousarray(np.asarray(pose_3d_pred, dtype=np.float32))
    ref = np.ascontiguousarray(np.asarray(pose_3d_ref, dtype=np.float32))
    mask = np.ascontiguousarray(np.asarray(valid_mask)).astype(np.uint8)
    eye = np.eye(P, dtype=ml_dtypes.bfloat16)
    in_maps = []
    for c in range(N_CORES):
        sl = slice(c * B_CORE, (c + 1) * B_CORE)
        in_maps.append(
            {
                "pred": pred[sl].reshape(T, P, R * ROW),
                "ref": ref[sl].reshape(T, P, R * ROW),
                "mask": mask[sl].reshape(T, P, R * KP),
                "eye": eye,
            }
        )
    return in_maps


def kernel(pose_3d_pred, pose_3d_ref, valid_mask, _trace=False):
    from concourse.bass_utils import run_bass_kernel_spmd

    nc = _get_nc()
    in_maps = _make_in_maps(pose_3d_pred, pose_3d_ref, valid_mask)
    res = run_bass_kernel_spmd(nc, in_maps, list(range(N_CORES)), trace=_trace)
    num = 0.0
    den = 0.0
    for i in range(N_CORES):
        o = res.results[i]["out"].astype(np.float64)
        num += o[:, 0].sum()
        den += o[:, 1].sum()
    out = np.float32(num / den)
    if _trace:
        return out, res
    return out
